# revision 3
# baseline (speedup 1.0000x reference)
"""Trainium2 Bass kernel for nn_Attention_3934190044008.

Multi-head attention with additive bias and sigmoid gating:
  q = (q_x @ w_q) / 8, k = kv_x @ w_k, v = kv_x @ w_v   (8 heads x 64)
  a = softmax(q k^T + bias);  o = a @ v
  o = o * sigmoid(q_x @ w_g + b_g);  out = o @ w_o + b_o

Sharding: 16 (batch, head) pairs over 8 cores -> each core owns one batch
element and 2 heads, produces a partial [2048, 256] output contribution
(o_slice @ w_o rows); host sums the 4 partials per batch and adds b_o.

Device-side layout is "feature on partitions" (transposed): scores are
computed as S^T [k, q] so the softmax denominator rides the AV matmul via a
ones-column appended to V, and softmax-over-k never needs a partition-axis
reduction. All transposes are done on the host (numpy) when building the
per-core input maps.

v3: every matmul runs in f32r (tf32-class, 1 cycle/row; the PE rounds
internally, so f32 data can be DMA'd into f32r-declared tensors), and the
additive bias is shipped as bf16 (halves the dominant 33.5 MB/core DMA
stream; |bias| ~ 1 so the 2^-9 rounding is ~2e-3 on scores). The bias add
is routed per-tile either through the DVE (S+bias -> SBUF, exp from SBUF)
or through the PE as an accumulating bf16 identity matmul (exp straight
from PSUM), controlled by KRN_BIAS_PE.
"""

import os
import sys
import threading
from contextlib import ExitStack

import numpy as np
import ml_dtypes

_REPO = "/opt/trn_rl_repo"
if _REPO not in sys.path and os.path.isdir(_REPO):
    sys.path.insert(0, _REPO)

import concourse.bass as bass  # noqa: E402
import concourse.mybir as mybir  # noqa: E402
import concourse.tile as tile  # noqa: E402
from concourse import bacc  # noqa: E402
from concourse.bass_utils import run_bass_kernel_spmd  # noqa: E402

F32 = mybir.dt.float32
F32R = mybir.dt.float32r
BF16 = mybir.dt.bfloat16
BF16NP = ml_dtypes.bfloat16

B, SEQ, CQ = 2, 2048, 256
H, DH = 8, 64
HD = H * DH  # 512
N_CORES = 8
HPC = 2  # heads per core

# fraction of score tiles whose bias add runs on the PE (bf16 identity
# matmul) instead of the DVE: "none", "half", "all"
BIAS_PE = os.environ.get("KRN_BIAS_PE", "none")


def _bias_on_pe(h, kt, qb):
    if BIAS_PE == "all":
        return True
    if BIAS_PE == "half":
        return qb == 0
    return False


def build_nc():
    nc = bacc.Bacc("TRN2", target_bir_lowering=False, debug=False)

    qxT = nc.dram_tensor("qxT", [CQ, SEQ], F32R, kind="ExternalInput").ap()
    kvxT = nc.dram_tensor("kvxT", [CQ, SEQ], F32R, kind="ExternalInput").ap()
    biasT = nc.dram_tensor("biasT", [HPC, SEQ, SEQ], BF16, kind="ExternalInput").ap()
    wq = nc.dram_tensor("wq", [CQ, HPC * DH], F32R, kind="ExternalInput").ap()
    wk = nc.dram_tensor("wk", [CQ, HPC * DH], F32R, kind="ExternalInput").ap()
    wv = nc.dram_tensor("wv", [CQ, HPC * DH], F32, kind="ExternalInput").ap()
    wg = nc.dram_tensor("wg", [CQ, HPC * DH], F32R, kind="ExternalInput").ap()
    bg = nc.dram_tensor("bg", [HPC * DH, 1], F32, kind="ExternalInput").ap()
    wo = nc.dram_tensor("wo", [HPC * DH, CQ], F32R, kind="ExternalInput").ap()
    ident = nc.dram_tensor("ident", [128, 128], BF16, kind="ExternalInput").ap()
    # per-head unnormalized partials + softmax denominators; the division
    # and cross-core summation happen on the host after the gather
    outs_d = [nc.dram_tensor(f"out{h}", [SEQ, CQ], F32, kind="ExternalOutput").ap()
              for h in range(HPC)]
    rs_d = nc.dram_tensor("rs", [1, HPC, SEQ], F32, kind="ExternalOutput").ap()

    NKT = SEQ // 128  # 16 k-tiles
    P = 128

    with tile.TileContext(nc) as tc:
        with ExitStack() as ctx:
            singles = ctx.enter_context(tc.tile_pool(name="singles", bufs=1))

            # ---- resident SBUF tensors ----
            # weights first (tiny, they gate the first projection matmuls);
            # one strided DMA per weight to minimize ~600ns-per-issue
            # sequencer serialization at startup
            w_sbs = {}
            for name, src, dt in (("wk", wk, F32R), ("wq", wq, F32R),
                                  ("wv", wv, F32), ("wg", wg, F32R)):
                t = singles.tile([P, 2, P], dt, tag=f"w_{name}")
                eng = nc.sync if name in ("wk", "wv") else nc.scalar
                eng.dma_start(t, src.rearrange("(a p) c -> p a c", p=P))
                w_sbs[name] = t
            bg_sb = singles.tile([P, 1], F32)
            nc.sync.dma_start(bg_sb, bg)
            ident_sb = singles.tile([P, P], BF16)
            nc.scalar.dma_start(ident_sb, ident)
            wo_sb = singles.tile([DH, HPC, CQ], F32R)
            nc.scalar.dma_start(wo_sb, wo.rearrange("(h p) c -> p h c", p=DH))

            # inputs as 1 MB halves, K-path first (it gates the first matmuls)
            qxT_sb = singles.tile([P, 2, SEQ], F32R)
            kvxT_sb = singles.tile([P, 2, SEQ], F32R)
            for a in range(2):
                (nc.sync if a == 0 else nc.scalar).dma_start(
                    kvxT_sb[:, a, :], kvxT[a * P:(a + 1) * P, :])
            for a in range(2):
                (nc.sync if a == 0 else nc.scalar).dma_start(
                    qxT_sb[:, a, :], qxT[a * P:(a + 1) * P, :])

            KT_sb = singles.tile([P, SEQ], F32R)   # [2h x 64 d, k]
            QT_sb = singles.tile([P, SEQ], F32R)   # [2h x 64 d, q]
            GT_sb = singles.tile([P, SEQ], F32)    # gate, [2 heads x 64, q]
            V_sb = singles.tile([P, HPC, NKT, DH + 1], F32R)  # [k%128, h, kt, d|1]
            OG_sb = singles.tile([DH, HPC, SEQ], F32R)  # (o * g)^T, final lhsT
            rs_sb = singles.tile([1, HPC, SEQ], F32)   # softmax denominators
            ones_col = V_sb[:, :, :, DH:DH + 1].bitcast(F32)
            nc.vector.memset(ones_col, 1.0)

            # ---- stage B: projections (f32r; V's is fp32 because its
            # moving dim is only 128 where f32r runs 1/4 rate anyway) ----
            with tc.tile_pool(name="ppsum", bufs=2, space="PSUM") as ppool:
                for wt, x_sb, dst in ((w_sbs["wk"], kvxT_sb, KT_sb),
                                      (w_sbs["wq"], qxT_sb, QT_sb)):
                    for tt in range(SEQ // 512):
                        ps = ppool.tile([P, 512], F32, tag="proj")
                        nc.tensor.matmul(ps, wt[:, 0, :],
                                         x_sb[:, 0, bass.ts(tt, 512)],
                                         start=True, stop=False)
                        nc.tensor.matmul(ps, wt[:, 1, :],
                                         x_sb[:, 1, bass.ts(tt, 512)],
                                         start=False, stop=True)
                        nc.vector.tensor_copy(dst[:, bass.ts(tt, 512)], ps)
                # gate projection + sigmoid (+ b_g as per-partition bias)
                for tt in range(SEQ // 512):
                    ps = ppool.tile([P, 512], F32, tag="projg")
                    nc.tensor.matmul(ps, w_sbs["wg"][:, 0, :],
                                     qxT_sb[:, 0, bass.ts(tt, 512)],
                                     start=True, stop=False)
                    nc.tensor.matmul(ps, w_sbs["wg"][:, 1, :],
                                     qxT_sb[:, 1, bass.ts(tt, 512)],
                                     start=False, stop=True)
                    nc.scalar.activation(GT_sb[:, bass.ts(tt, 512)], ps,
                                         mybir.ActivationFunctionType.Sigmoid,
                                         bias=bg_sb)
                # V projection: out rows = tokens(k), cols = 2 heads x 64
                for kt in range(NKT):
                    ps = ppool.tile([P, P], F32, tag="vproj")
                    nc.tensor.matmul(ps, kvxT_sb[:, 0, bass.ts(kt, P)].bitcast(F32),
                                     w_sbs["wv"][:, 0, :],
                                     start=True, stop=False)
                    nc.tensor.matmul(ps, kvxT_sb[:, 1, bass.ts(kt, P)].bitcast(F32),
                                     w_sbs["wv"][:, 1, :],
                                     start=False, stop=True)
                    nc.vector.tensor_copy(V_sb[:, 0, kt, 0:DH], ps[:, 0:DH])
                    nc.vector.tensor_copy(V_sb[:, 1, kt, 0:DH], ps[:, DH:2 * DH])

            # ---- stage C: attention ----
            # kt-outer / q-block-inner: one contiguous 512 KB bf16 bias DMA
            # per (head, k-tile); both q-block OT accumulators stay live in
            # PSUM (2 x 2 banks) next to the double-buffered S tiles (2 x 2).
            QB = 1024
            NQB = SEQ // QB
            with tc.tile_pool(name="otpsum", bufs=2, space="PSUM") as otpool, \
                 tc.tile_pool(name="spsum", bufs=2, space="PSUM") as spool, \
                 tc.tile_pool(name="biasp", bufs=10) as biaspool, \
                 tc.tile_pool(name="sbp", bufs=4) as sbpool, \
                 tc.tile_pool(name="ep", bufs=6) as epool:
                for h in range(HPC):
                    hsl = slice(h * DH, (h + 1) * DH)
                    OTs = [otpool.tile([DH + 1, QB], F32, name=f"OT{h}_{qb}",
                                       tag="ot")
                           for qb in range(NQB)]
                    for kt in range(NKT):
                        bias_sb = biaspool.tile([P, SEQ], BF16)
                        # spread bias transfers over three DMA paths (two
                        # HWDGE rings + SWDGE) so they overlap instead of
                        # serializing on one FIFO
                        dma_eng = (nc.sync, nc.scalar, nc.gpsimd)[kt % 3]
                        dma_eng.dma_start(bias_sb, biasT[h, bass.ts(kt, P), :])
                        for qb in range(NQB):
                            q0 = qb * QB
                            S = spool.tile([P, QB], F32, tag="s")
                            on_pe = _bias_on_pe(h, kt, qb)
                            for j in range(2):
                                nc.tensor.matmul(
                                    S[:, bass.ts(j, 512)],
                                    KT_sb[hsl, bass.ts(kt, P)],
                                    QT_sb[hsl, bass.ds(q0 + j * 512, 512)],
                                    start=True, stop=not on_pe)
                            E = epool.tile([P, QB], F32R)
                            if on_pe:
                                for j in range(2):
                                    nc.tensor.matmul(
                                        S[:, bass.ts(j, 512)],
                                        ident_sb,
                                        bias_sb[:, bass.ds(q0 + j * 512, 512)],
                                        start=False, stop=True)
                                nc.scalar.activation(
                                    E, S, mybir.ActivationFunctionType.Exp)
                            else:
                                SB = sbpool.tile([P, QB], F32, tag="SB")
                                nc.vector.tensor_add(SB, S,
                                                     bias_sb[:, bass.ds(q0, QB)])
                                nc.scalar.activation(
                                    E, SB, mybir.ActivationFunctionType.Exp)
                            for j in range(2):
                                nc.tensor.matmul(
                                    OTs[qb][:, bass.ts(j, 512)],
                                    V_sb[:, h, kt, :],
                                    E[:, bass.ts(j, 512)],
                                    start=(kt == 0), stop=(kt == NKT - 1))
                    # gate (unnormalized) and stash the exp-sum row; the
                    # softmax division happens on the host
                    for qb in range(NQB):
                        q0 = qb * QB
                        OT = OTs[qb]
                        # on the last head the exp-sum copy rides ACT so
                        # the DVE epilogue chain stays under the ~3.4us HAM
                        # re-throttle window before the output projections
                        if h == HPC - 1:
                            nc.scalar.copy(rs_sb[:, h, bass.ds(q0, QB)],
                                           OT[DH:DH + 1, :])
                        else:
                            nc.vector.tensor_copy(rs_sb[:, h, bass.ds(q0, QB)],
                                                  OT[DH:DH + 1, :])
                        nc.vector.tensor_mul(OG_sb[:, h, bass.ds(q0, QB)],
                                             GT_sb[hsl, bass.ds(q0, QB)],
                                             OT[0:DH, :])

                # ---- stage D: per-head output projections (partials) ----
                # inside the attention pool scope, with PSUM riding the
                # S-pool slots: no pool-close barrier, so head 0's finals
                # (ready since mid-kernel) start the moment an S slot frees
                # after the last exp, covering head 1's epilogue on DVE and
                # keeping the PE clock warm into the tail.
                for h in range(HPC):
                    for tt in range(SEQ // P):
                        ps = spool.tile([P, CQ], F32, tag="s", name="fin_ps")
                        nc.tensor.matmul(ps, OG_sb[:, h, bass.ts(tt, P)],
                                         wo_sb[:, h, :], start=True, stop=True)
                        o_sb = sbpool.tile([P, CQ], F32, tag="SB",
                                           name="fin_osb")
                        nc.vector.tensor_copy(o_sb, ps)
                        eng = nc.sync if tt % 2 == 0 else nc.scalar
                        eng.dma_start(outs_d[h][bass.ts(tt, P), :], o_sb)

            nc.sync.dma_start(rs_d, rs_sb)

    nc.compile()
    return nc


_NC = None
_NC_LOCK = threading.Lock()


def _get_nc():
    global _NC
    with _NC_LOCK:
        if _NC is None:
            _NC = build_nc()
        return _NC


def make_in_maps(q_x, kv_x, bias, w_q, w_k, w_v, w_g, b_g, w_o, b_o):
    del b_o  # added on the host after the gather
    q_x = np.asarray(q_x, dtype=np.float32)
    kv_x = np.asarray(kv_x, dtype=np.float32)
    bias = np.asarray(bias, dtype=np.float32)
    w_q = np.asarray(w_q, dtype=np.float32) * np.float32(0.125)  # fold 1/sqrt(64)
    w_k = np.asarray(w_k, dtype=np.float32)
    w_v = np.asarray(w_v, dtype=np.float32)
    w_g = np.asarray(w_g, dtype=np.float32)
    b_g = np.asarray(b_g, dtype=np.float32)
    w_o = np.asarray(w_o, dtype=np.float32)
    ident = np.eye(128, dtype=BF16NP)

    in_maps = []
    for c in range(N_CORES):
        b = c // (N_CORES // B)
        h0 = HPC * (c % (N_CORES // B))
        cols = slice(h0 * DH, (h0 + HPC) * DH)
        in_maps.append({
            "qxT": np.ascontiguousarray(q_x[b].T),
            "kvxT": np.ascontiguousarray(kv_x[b].T),
            "biasT": np.ascontiguousarray(
                bias[b, h0:h0 + HPC].swapaxes(1, 2).astype(BF16NP)),
            "wq": np.ascontiguousarray(w_q[:, cols]),
            "wk": np.ascontiguousarray(w_k[:, cols]),
            "wv": np.ascontiguousarray(w_v[:, cols]),
            "wg": np.ascontiguousarray(w_g[:, cols]),
            "bg": np.ascontiguousarray(b_g[cols].reshape(HPC * DH, 1)),
            "wo": np.ascontiguousarray(w_o[cols, :]),
            "ident": ident,
        })
    return in_maps


def gather_output(results, b_o):
    full = np.zeros((B, SEQ, CQ), dtype=np.float32)
    for c in range(N_CORES):
        b = c // (N_CORES // B)
        rs = results[c]["rs"][0]
        for h in range(HPC):
            full[b] += results[c][f"out{h}"] / rs[h][:, None]
    full += np.asarray(b_o, dtype=np.float32)
    return full


def kernel(**inputs):
    nc = _get_nc()
    in_maps = make_in_maps(**inputs)
    res = run_bass_kernel_spmd(nc, in_maps, core_ids=list(range(N_CORES)))
    return gather_output(res.results, inputs["b_o"])


# revision 4
# speedup vs baseline: 1.2556x; 1.2556x over previous
"""Trainium2 Bass kernel for nn_Attention_3934190044008.

Multi-head attention with additive bias and sigmoid gating:
  q = (q_x @ w_q) / 8, k = kv_x @ w_k, v = kv_x @ w_v   (8 heads x 64)
  a = softmax(q k^T + bias);  o = a @ v
  o = o * sigmoid(q_x @ w_g + b_g);  out = o @ w_o + b_o

Sharding: 16 (batch, head) pairs over 8 cores -> each core owns one batch
element and 2 heads, produces a partial [2048, 256] output contribution
(o_slice @ w_o rows); host sums the 4 partials per batch and adds b_o.

Device-side layout is "feature on partitions" (transposed): scores are
computed as S^T [k, q] so the softmax denominator rides the AV matmul via a
ones-column appended to V, and softmax-over-k never needs a partition-axis
reduction. All transposes are done on the host (numpy).

v4: q-block-outer loop with both heads interleaved per k-tile so the two
heads' 64-contract QK matmuls land in disjoint PE row groups
(tile_position auto-derived from base partition) and run concurrently;
same packing for the two heads' output projections. All matmuls are f32r
(tf32-class, 1 cycle/row); the additive bias ships as bf16 (halves the
33.5 MB/core stream) and is added on the PE as an accumulating bf16
identity matmul, which keeps the PE stream dense enough to hold the HAM
clock gate at 2.4 GHz (sparse PE streams re-throttle to 1.2 GHz and the
whole kernel inflates). Input DMAs are chunked so projections start
early, and half the output projections interleave into the second
q-block to shorten the tail.
"""

import os
import sys
import threading
from contextlib import ExitStack

import numpy as np
import ml_dtypes

_REPO = "/opt/trn_rl_repo"
if _REPO not in sys.path and os.path.isdir(_REPO):
    sys.path.insert(0, _REPO)

import concourse.bass as bass  # noqa: E402
import concourse.mybir as mybir  # noqa: E402
import concourse.tile as tile  # noqa: E402
from concourse import bacc  # noqa: E402
from concourse.bass_utils import run_bass_kernel_spmd  # noqa: E402

F32 = mybir.dt.float32
F32R = mybir.dt.float32r
BF16 = mybir.dt.bfloat16
BF16NP = ml_dtypes.bfloat16

B, SEQ, CQ = 2, 2048, 256
H, DH = 8, 64
HD = H * DH  # 512
N_CORES = 8
HPC = 2  # heads per core

# which heads' bias adds ride the PE as identity matmuls ("all"/"h0"/"none");
# the rest go through the DVE. "all" keeps the PE stream densest.
BIAS_PE = os.environ.get("KRN_BIAS_PE", "all")


def _bias_on_pe(h):
    if BIAS_PE == "all":
        return True
    if BIAS_PE == "h0":
        return h == 0
    return False


def build_nc():
    nc = bacc.Bacc("TRN2", target_bir_lowering=False, debug=False)

    qxT = nc.dram_tensor("qxT", [CQ, SEQ], F32R, kind="ExternalInput").ap()
    kvxT = nc.dram_tensor("kvxT", [CQ, SEQ], F32R, kind="ExternalInput").ap()
    biasT = nc.dram_tensor("biasT", [HPC, SEQ, SEQ], BF16, kind="ExternalInput").ap()
    wq = nc.dram_tensor("wq", [CQ, HPC * DH], F32R, kind="ExternalInput").ap()
    wk = nc.dram_tensor("wk", [CQ, HPC * DH], F32R, kind="ExternalInput").ap()
    wv = nc.dram_tensor("wv", [CQ, HPC * DH], F32, kind="ExternalInput").ap()
    wg = nc.dram_tensor("wg", [CQ, HPC * DH], F32R, kind="ExternalInput").ap()
    bg = nc.dram_tensor("bg", [HPC * DH, 1], F32, kind="ExternalInput").ap()
    wo = nc.dram_tensor("wo", [HPC * DH, CQ], F32R, kind="ExternalInput").ap()
    ident = nc.dram_tensor("ident", [128, 128], BF16, kind="ExternalInput").ap()
    outs_d = [nc.dram_tensor(f"out{h}", [SEQ, CQ], F32, kind="ExternalOutput").ap()
              for h in range(HPC)]
    rs_d = nc.dram_tensor("rs", [1, HPC, SEQ], F32, kind="ExternalOutput").ap()

    NKT = SEQ // 128  # 16 k-tiles
    P = 128
    QB = 1024
    NQB = SEQ // QB
    NTT = SEQ // P  # 16 output-projection chunks

    with tile.TileContext(nc) as tc:
        with ExitStack() as ctx:
            singles = ctx.enter_context(tc.tile_pool(name="singles", bufs=1))

            # ---- resident SBUF tensors ----
            # weights first (tiny, they gate the first projection matmuls)
            w_sbs = {}
            for name, src, dt in (("wk", wk, F32R), ("wq", wq, F32R),
                                  ("wv", wv, F32), ("wg", wg, F32R)):
                t = singles.tile([P, 2, P], dt, tag=f"w_{name}")
                eng = nc.sync if name in ("wk", "wv") else nc.scalar
                eng.dma_start(t, src.rearrange("(a p) c -> p a c", p=P))
                w_sbs[name] = t
            bg_sb = singles.tile([P, 1], F32)
            nc.sync.dma_start(bg_sb, bg)
            ident_sb = singles.tile([P, P], BF16)
            nc.scalar.dma_start(ident_sb, ident)
            wo_sb = singles.tile([P, CQ], F32R)  # heads stacked on partitions
            nc.scalar.dma_start(wo_sb, wo)

            # inputs in 512-column chunks so the first projections can start
            # after ~0.5 MB instead of waiting for the full 2 MB tensor
            qxT_sb = singles.tile([P, 2, SEQ], F32R)
            kvxT_sb = singles.tile([P, 2, SEQ], F32R)
            for tt in range(4):
                for a in range(2):
                    nc.sync.dma_start(
                        kvxT_sb[:, a, bass.ts(tt, 512)],
                        kvxT[a * P:(a + 1) * P, bass.ts(tt, 512)])
                    nc.scalar.dma_start(
                        qxT_sb[:, a, bass.ts(tt, 512)],
                        qxT[a * P:(a + 1) * P, bass.ts(tt, 512)])

            KT_sb = singles.tile([P, SEQ], F32R)   # [2h x 64 d, k]
            QT_sb = singles.tile([P, SEQ], F32R)   # [2h x 64 d, q]
            GT_sb = singles.tile([P, SEQ], F32)    # gate, [2 heads x 64, q]
            V_sb = singles.tile([P, HPC, NKT, DH + 1], F32R)  # [k%128, h, kt, d|1]
            OG_sb = singles.tile([P, SEQ], F32R)   # (o * g)^T, heads stacked
            rs_sb = singles.tile([1, HPC, SEQ], F32)   # softmax denominators
            ones_col = V_sb[:, :, :, DH:DH + 1].bitcast(F32)
            nc.vector.memset(ones_col, 1.0)

            # ---- single shared PSUM layout: OT pool (4 banks) + S pool
            # (4 banks). Projections and output-projection tiles ride the
            # S pool slots, so there is no pool-close barrier anywhere. ----
            with tc.tile_pool(name="otpsum", bufs=2, space="PSUM") as otpool, \
                 tc.tile_pool(name="spsum", bufs=2, space="PSUM") as spool, \
                 tc.tile_pool(name="biasp", bufs=12) as biaspool, \
                 tc.tile_pool(name="sbp", bufs=4) as sbpool, \
                 tc.tile_pool(name="ep", bufs=6) as epool:

                # ---- stage B: projections (f32r; V's is fp32 because its
                # moving dim is only 128 where f32r runs 1/4 rate anyway).
                # K/Q tt-interleaved so attention can start after tt0/tt1;
                # V per-kt follows; G (only needed by the epilogue) last.
                def proj_kq(wt, x_sb, dst, tt):
                    ps = spool.tile([P, 512], F32, tag="s", name="proj")
                    nc.tensor.matmul(ps, wt[:, 0, :],
                                     x_sb[:, 0, bass.ts(tt, 512)],
                                     start=True, stop=False)
                    nc.tensor.matmul(ps, wt[:, 1, :],
                                     x_sb[:, 1, bass.ts(tt, 512)],
                                     start=False, stop=True)
                    nc.vector.tensor_copy(dst[:, bass.ts(tt, 512)], ps)

                for tt in range(4):
                    proj_kq(w_sbs["wk"], kvxT_sb, KT_sb, tt)
                    proj_kq(w_sbs["wq"], qxT_sb, QT_sb, tt)
                for kt in range(NKT):
                    ps = spool.tile([P, P], F32, tag="s", name="vproj")
                    nc.tensor.matmul(ps,
                                     kvxT_sb[:, 0, bass.ts(kt, P)].bitcast(F32),
                                     w_sbs["wv"][:, 0, :],
                                     start=True, stop=False)
                    nc.tensor.matmul(ps,
                                     kvxT_sb[:, 1, bass.ts(kt, P)].bitcast(F32),
                                     w_sbs["wv"][:, 1, :],
                                     start=False, stop=True)
                    nc.vector.tensor_copy(V_sb[:, 0, kt, 0:DH], ps[:, 0:DH])
                    nc.vector.tensor_copy(V_sb[:, 1, kt, 0:DH], ps[:, DH:2 * DH])
                for tt in range(4):
                    ps = spool.tile([P, 512], F32, tag="s", name="projg")
                    nc.tensor.matmul(ps, w_sbs["wg"][:, 0, :],
                                     qxT_sb[:, 0, bass.ts(tt, 512)],
                                     start=True, stop=False)
                    nc.tensor.matmul(ps, w_sbs["wg"][:, 1, :],
                                     qxT_sb[:, 1, bass.ts(tt, 512)],
                                     start=False, stop=True)
                    nc.scalar.activation(GT_sb[:, bass.ts(tt, 512)], ps,
                                         mybir.ActivationFunctionType.Sigmoid,
                                         bias=bg_sb)

                # one output-projection chunk: both heads' [64,128] lhsT sit
                # on partitions 0-63 / 64-127 -> row groups (0,0)/(64,0),
                # so the pair runs concurrently in the PE array
                def fin_chunk(tt):
                    for h in range(HPC):
                        ps = spool.tile([P, CQ], F32, tag="s",
                                        name=f"fin{h}_{tt}")
                        nc.tensor.matmul(ps,
                                         OG_sb[h * DH:(h + 1) * DH,
                                               bass.ts(tt, P)],
                                         wo_sb[h * DH:(h + 1) * DH, :],
                                         start=True, stop=True)
                        o_sb = sbpool.tile([P, CQ], F32, tag="SB",
                                           name=f"fino{h}_{tt}")
                        nc.vector.tensor_copy(o_sb, ps)
                        eng = nc.sync if (tt + h) % 2 == 0 else nc.scalar
                        eng.dma_start(outs_d[h][bass.ts(tt, P), :], o_sb)

                # ---- stage C: attention, q-block outer ----
                ndma = 0
                for qb in range(NQB):
                    q0 = qb * QB
                    OTs = [otpool.tile([DH + 1, QB], F32, name=f"OT{qb}_{h}",
                                       tag="ot") for h in range(HPC)]
                    for kt in range(NKT):
                        bias_t = []
                        for h in range(HPC):
                            bt = biaspool.tile([P, QB], BF16)
                            # first transfers ride SWDGE (gpsimd) so the two
                            # HWDGE rings stay clear for the input chunks
                            dma_eng = (nc.gpsimd, nc.sync, nc.scalar)[ndma % 3]
                            ndma += 1
                            dma_eng.dma_start(
                                bt, biasT[h, bass.ts(kt, P),
                                          bass.ds(q0, QB)])
                            bias_t.append(bt)
                        Ss = [spool.tile([P, QB], F32, tag="s",
                                         name=f"S{qb}_{kt}_{h}")
                              for h in range(HPC)]
                        # packed QK: adjacent instructions on disjoint row
                        # groups overlap in the array
                        for j in range(2):
                            for h in range(HPC):
                                hsl = slice(h * DH, (h + 1) * DH)
                                nc.tensor.matmul(
                                    Ss[h][:, bass.ts(j, 512)],
                                    KT_sb[hsl, bass.ts(kt, P)],
                                    QT_sb[hsl, bass.ds(q0 + j * 512, 512)],
                                    start=True, stop=not _bias_on_pe(h))
                        Es = []
                        for h in range(HPC):
                            E = epool.tile([P, QB], F32R)
                            if _bias_on_pe(h):
                                for j in range(2):
                                    nc.tensor.matmul(
                                        Ss[h][:, bass.ts(j, 512)],
                                        ident_sb,
                                        bias_t[h][:, bass.ts(j, 512)],
                                        start=False, stop=True)
                                nc.scalar.activation(
                                    E, Ss[h], mybir.ActivationFunctionType.Exp)
                            else:
                                SB = sbpool.tile([P, QB], F32, tag="SB")
                                nc.vector.tensor_add(SB, Ss[h], bias_t[h])
                                nc.scalar.activation(
                                    E, SB, mybir.ActivationFunctionType.Exp)
                            Es.append(E)
                        for h in range(HPC):
                            for j in range(2):
                                nc.tensor.matmul(
                                    OTs[h][:, bass.ts(j, 512)],
                                    V_sb[:, h, kt, :],
                                    Es[h][:, bass.ts(j, 512)],
                                    start=(kt == 0), stop=(kt == NKT - 1))
                        # interleave the first q-block's output projections
                        # into the second q-block's stream: keeps the PE
                        # dense and shortens the tail
                        if qb == 1 and kt % 2 == 1:
                            fin_chunk(kt // 2)
                    # epilogue for this q-block, both heads
                    for h in range(HPC):
                        hsl = slice(h * DH, (h + 1) * DH)
                        OT = OTs[h]
                        if qb == NQB - 1 and h == HPC - 1:
                            nc.scalar.copy(rs_sb[:, h, bass.ds(q0, QB)],
                                           OT[DH:DH + 1, :])
                        else:
                            nc.vector.tensor_copy(rs_sb[:, h, bass.ds(q0, QB)],
                                                  OT[DH:DH + 1, :])
                        nc.vector.tensor_mul(OG_sb[hsl, bass.ds(q0, QB)],
                                             GT_sb[hsl, bass.ds(q0, QB)],
                                             OT[0:DH, :])

                # ---- stage D: remaining output projections (tt 8..15) ----
                for tt in range(NTT // 2, NTT):
                    fin_chunk(tt)

            nc.sync.dma_start(rs_d, rs_sb)

    nc.compile()
    return nc


_NC = None
_NC_LOCK = threading.Lock()


def _get_nc():
    global _NC
    with _NC_LOCK:
        if _NC is None:
            _NC = build_nc()
        return _NC


def make_in_maps(q_x, kv_x, bias, w_q, w_k, w_v, w_g, b_g, w_o, b_o):
    del b_o  # added on the host after the gather
    q_x = np.asarray(q_x, dtype=np.float32)
    kv_x = np.asarray(kv_x, dtype=np.float32)
    bias = np.asarray(bias, dtype=np.float32)
    w_q = np.asarray(w_q, dtype=np.float32) * np.float32(0.125)  # fold 1/sqrt(64)
    w_k = np.asarray(w_k, dtype=np.float32)
    w_v = np.asarray(w_v, dtype=np.float32)
    w_g = np.asarray(w_g, dtype=np.float32)
    b_g = np.asarray(b_g, dtype=np.float32)
    w_o = np.asarray(w_o, dtype=np.float32)
    ident = np.eye(128, dtype=BF16NP)

    in_maps = []
    for c in range(N_CORES):
        b = c // (N_CORES // B)
        h0 = HPC * (c % (N_CORES // B))
        cols = slice(h0 * DH, (h0 + HPC) * DH)
        in_maps.append({
            "qxT": np.ascontiguousarray(q_x[b].T),
            "kvxT": np.ascontiguousarray(kv_x[b].T),
            "biasT": np.ascontiguousarray(
                bias[b, h0:h0 + HPC].swapaxes(1, 2).astype(BF16NP)),
            "wq": np.ascontiguousarray(w_q[:, cols]),
            "wk": np.ascontiguousarray(w_k[:, cols]),
            "wv": np.ascontiguousarray(w_v[:, cols]),
            "wg": np.ascontiguousarray(w_g[:, cols]),
            "bg": np.ascontiguousarray(b_g[cols].reshape(HPC * DH, 1)),
            "wo": np.ascontiguousarray(w_o[cols, :]),
            "ident": ident,
        })
    return in_maps


def gather_output(results, b_o):
    full = np.zeros((B, SEQ, CQ), dtype=np.float32)
    for c in range(N_CORES):
        b = c // (N_CORES // B)
        rs = results[c]["rs"][0]
        for h in range(HPC):
            full[b] += results[c][f"out{h}"] / rs[h][:, None]
    full += np.asarray(b_o, dtype=np.float32)
    return full


def kernel(**inputs):
    nc = _get_nc()
    in_maps = make_in_maps(**inputs)
    res = run_bass_kernel_spmd(nc, in_maps, core_ids=list(range(N_CORES)))
    return gather_output(res.results, inputs["b_o"])
